# revision 15
# baseline (speedup 1.0000x reference)
"""GRU decoder (AutoEncoder) Trainium2 kernel — 8 NeuronCores, vocab-sharded.

Per core (all 8 symmetric; core i owns vocab rows [4000i, 4000(i+1))):
  - PSUM pre-fill (off critical path, batched 16 steps at a time, N=128
    matmuls): gate pre-activations gx = W_ih @ x^T + biases accumulated
    directly into PSUM banks, step-major layout [16 steps x 32 cols].
  - 511 sequential GRU steps: 12 small matmuls accumulate W_hh @ h on top of
    the pre-filled PSUM; sigmoid/tanh on ACT straight from PSUM; remaining
    gate math on DVE; h' written directly as bf16 into a history buffer that
    is also the projection's lhsT.
  - Every 16 steps: project 128 rows H^T @ W_out_shard (+ b_out via a
    ones-row matmul), PSUM->SBUF copy on ACT, DMA out.

Gate slots: r = rows 0:256, z = 256:512, n = 512:768 of the 3H dim.
b_ih+b_hh pre-filled for r,z; the n-gate PSUM region holds W_hh_n h + b_hh_n
while xn(+b_ih_n) sits in its own region, matching the reference's
n = tanh(xn + r*(W_hh h + b_hh)_n).  Blend: h' = n*(1-z) + z*h with (1-z),
z*h computed while ACT runs tanh.
"""

import numpy as np
import ml_dtypes

B = 8
T = 512
V = 32000
D = 256
H = 256
TT = T - 1            # 511 decode steps
NCORES = 8
VS = V // NCORES      # 4000 vocab rows per core
NT = 8                # vocab tiles per core
NSL = VS // NT        # 500 columns per vocab tile
RPT = 16              # steps per block (16*8=128 rows / psum bank)

_bf16 = ml_dtypes.bfloat16

_CACHE = {}


def _build(tt_steps=TT):
    """Build + compile the single-NEFF SPMD kernel. Returns the Bass object."""
    import concourse.mybir as mybir
    from concourse import bacc
    from concourse.tile import TileContext
    from concourse.bass import ds, ts

    f32 = mybir.dt.float32
    bf16 = mybir.dt.bfloat16
    AF = mybir.ActivationFunctionType
    OP = mybir.AluOpType

    rows = tt_steps * B
    n_rt = (rows + 127) // 128  # 16-step blocks (= projection row-tiles)

    nc = bacc.Bacc("TRN2", target_bir_lowering=False, debug=False,
                   num_devices=NCORES)

    xT_d = nc.dram_tensor("xT", [2, 128, rows], bf16, kind="ExternalInput").ap()
    wih_d = nc.dram_tensor("wih", [2, 128, 768], bf16, kind="ExternalInput").ap()
    whh_d = nc.dram_tensor("whh", [2, 128, 768], bf16, kind="ExternalInput").ap()
    wout_d = nc.dram_tensor("wout", [2, 128, VS], bf16, kind="ExternalInput").ap()
    brz_d = nc.dram_tensor("brz", [1, 512], bf16, kind="ExternalInput").ap()
    bhn_d = nc.dram_tensor("bhn", [1, 256], bf16, kind="ExternalInput").ap()
    bxn_d = nc.dram_tensor("bxn", [1, 256], bf16, kind="ExternalInput").ap()
    bout_d = nc.dram_tensor("bout", [1, VS], bf16, kind="ExternalInput").ap()
    out_d = nc.dram_tensor("out", [rows, VS], f32, kind="ExternalOutput").ap()

    with TileContext(nc) as tc:
        with (
            tc.tile_pool(name="singles", bufs=1) as singles,
            tc.tile_pool(name="work", bufs=3) as work,
            tc.tile_pool(name="stage", bufs=3) as stagep,
            tc.tile_pool(name="gpsum", bufs=2, space="PSUM") as gpsum,
            tc.tile_pool(name="ppsum", bufs=3, space="PSUM") as ppsum,
        ):
            # ---- resident tensors ----
            xT_sb = singles.tile([128, 2, rows], bf16, tag="xT")
            wih_sb = singles.tile([128, 2, 768], bf16, tag="wih")
            whh_sb = singles.tile([128, 2, 768], bf16, tag="whh")
            wout_sb = singles.tile([128, 2, VS], bf16, tag="wout")
            brz_sb = singles.tile([1, 512], bf16, tag="brz")
            bhn_sb = singles.tile([1, 256], bf16, tag="bhn")
            bxn_sb = singles.tile([1, 256], bf16, tag="bxn")
            bout_sb = singles.tile([1, VS], bf16, tag="bout")
            ones_sb = singles.tile([1, 128], bf16, tag="ones")
            z8 = singles.tile([128, 2, B], bf16, tag="z8")
            hh = [
                singles.tile([128, 2, min(128, rows - 128 * R)], bf16,
                             tag=f"hh{R}", name=f"hh{R}")
                for R in range(n_rt)
            ]

            for k in range(2):
                nc.sync.dma_start(out=xT_sb[:, k, :], in_=xT_d[k])
                nc.sync.dma_start(out=wih_sb[:, k, :], in_=wih_d[k])
                nc.sync.dma_start(out=whh_sb[:, k, :], in_=whh_d[k])
                nc.sync.dma_start(out=wout_sb[:, k, :], in_=wout_d[k])
            nc.sync.dma_start(out=brz_sb[:], in_=brz_d)
            nc.sync.dma_start(out=bhn_sb[:], in_=bhn_d)
            nc.sync.dma_start(out=bxn_sb[:], in_=bxn_d)
            nc.sync.dma_start(out=bout_sb[:], in_=bout_d)
            nc.vector.memset(ones_sb[:], 1.0)
            nc.vector.memset(z8[:], 0.0)

            # ---- HAM warmup: ~5us of dense matmuls so the PE clock-gate
            # opens to 2.4 GHz before the recurrence; per-step PE idle gaps
            # (<2us) never re-throttle it afterwards.
            warm = ppsum.tile([128, 512], f32, tag="proj", name="warmps")
            for w in range(20):
                nc.tensor.matmul(
                    warm[:, :512], whh_sb[:, 0, 0:128], whh_sb[:, 0, 0:512],
                    start=(w == 0), stop=(w == 19), skip_group_check=True,
                )

            # ---- PSUM pre-fill for one 16-step block ----
            # rzp: [16 steps x (r0 r1 z0 z1)] step-major; gxn: [16 x (gn0 gn1
            # xn0 xn1)].  Returns the two psum tiles.
            def emit_prefill(bk):
                nb = min(RPT, tt_steps - RPT * bk)  # steps in block
                c0 = 128 * bk                       # first row of block
                cn = nb * B
                rzp = gpsum.tile([128, 512], f32, tag="rzp")
                gxn = gpsum.tile([128, 512], f32, tag="gxn")
                vrz = rzp[:, :32 * nb].rearrange("p (t g) -> p t g", g=32)
                vgx = gxn[:, :32 * nb].rearrange("p (t g) -> p t g", g=32)
                ops = []
                for s in range(4):  # r0 r1 z0 z1: W_ih_rz x + (b_ih+b_hh)
                    for k in range(2):
                        ops.append((vrz[:, :, ds(8 * s, 8)],
                                    wih_sb[:, k, ts(s, 128)],
                                    xT_sb[:, k, ds(c0, cn)], k == 0, False))
                    ops.append((vrz[:, :, ds(8 * s, 8)],
                                brz_sb[:, ts(s, 128)],
                                ones_sb[:, :cn], False, False))
                for s in range(2):  # xn region: W_ih_n x + b_ih_n (complete)
                    for k in range(2):
                        ops.append((vgx[:, :, ds(16 + 8 * s, 8)],
                                    wih_sb[:, k, ts(4 + s, 128)],
                                    xT_sb[:, k, ds(c0, cn)], k == 0, False))
                    ops.append((vgx[:, :, ds(16 + 8 * s, 8)],
                                bxn_sb[:, ts(s, 128)],
                                ones_sb[:, :cn], False, True))
                closures = []
                for (o, l, r_, st, sp) in ops:
                    def mk(o=o, l=l, r_=r_, st=st, sp=sp):
                        nc.tensor.matmul(o, l, r_, start=st, stop=sp,
                                         skip_group_check=True)
                    closures.append(mk)
                return rzp, gxn, closures

            proj_q = []

            def emit_proj(R):
                h_t = hh[R]
                mr = h_t.shape[2]
                for ntile in range(NT):
                    # one closure per PE/ACT/DMA instruction so the queue can
                    # interleave projection work finely between steps
                    state = {}

                    def mk_mm(k, R=R, ntile=ntile, h_t=h_t, mr=mr, state=state):
                        if k == 0:
                            state["pp"] = ppsum.tile([128, 512], f32,
                                                     tag="proj", name="projpp")
                        nc.tensor.matmul(
                            state["pp"][:mr, :NSL], h_t[:, k, :],
                            wout_sb[:, k, ds(ntile * NSL, NSL)],
                            start=(k == 0), stop=False,
                        )

                    def mk_bias(R=R, ntile=ntile, mr=mr, state=state):
                        nc.tensor.matmul(
                            state["pp"][:mr, :NSL], ones_sb[:, :mr],
                            bout_sb[:, ds(ntile * NSL, NSL)],
                            start=False, stop=True,
                        )

                    def mk_out(R=R, ntile=ntile, mr=mr, state=state):
                        st = stagep.tile([128, NSL], f32, tag="stage")
                        nc.scalar.copy(st[:mr], state["pp"][:mr, :NSL])
                        nc.sync.dma_start(
                            out=out_d[ds(128 * R, mr), ds(ntile * NSL, NSL)],
                            in_=st[:mr],
                        )
                    proj_q.append(lambda mk_mm=mk_mm: mk_mm(0))
                    proj_q.append(lambda mk_mm=mk_mm: mk_mm(1))
                    proj_q.append(mk_bias)
                    proj_q.append(mk_out)

            # block 0 pre-fill up front
            rzp, gxn, pf = emit_prefill(0)
            for f in pf:
                f()
            nxt = None      # next block's psum tiles
            pf_q = []       # pending pre-fill matmuls for next block

            for t in range(tt_steps):
                j = t % RPT
                if t == 0:
                    rhs = z8
                    roff = 0
                else:
                    rhs = hh[(t - 1) // RPT]
                    roff = ((t - 1) % RPT) * B

                # W_hh @ h accumulated onto pre-filled psum
                for s in range(4):
                    for k in range(2):
                        nc.tensor.matmul(
                            rzp[:, ds(32 * j + 8 * s, 8)],
                            whh_sb[:, k, ts(s, 128)],
                            rhs[:, k, ds(roff, B)],
                            start=False, stop=(k == 1), skip_group_check=True,
                        )
                for s in range(2):
                    for k in range(2):
                        nc.tensor.matmul(
                            gxn[:, ds(32 * j + 8 * s, 8)],
                            whh_sb[:, k, ts(4 + s, 128)],
                            rhs[:, k, ds(roff, B)],
                            start=(k == 0), stop=False, skip_group_check=True,
                        )
                    # += b_hh_n broadcast (completes the gn region group)
                    nc.tensor.matmul(
                        gxn[:, ds(32 * j + 8 * s, 8)],
                        bhn_sb[:, ts(s, 128)], ones_sb[:, :B],
                        start=False, stop=True, skip_group_check=True,
                    )

                # split sigmoid: r-half unblocks tb as soon as the 4 r-tile
                # matmuls land; z-half overlaps the tb/u DVE work
                rz = work.tile([128, 32], f32, tag="rz")
                nc.scalar.activation(rz[:, 0:16], rzp[:, ds(32 * j, 16)],
                                     AF.Sigmoid)
                nc.scalar.activation(rz[:, 16:32], rzp[:, ds(32 * j + 16, 16)],
                                     AF.Sigmoid)

                tb = work.tile([128, 16], f32, tag="tb")
                nc.vector.tensor_mul(tb[:], rz[:, 0:16],
                                     gxn[:, ds(32 * j, 16)])
                u = work.tile([128, 16], f32, tag="u")
                nc.vector.tensor_add(u[:], tb[:], gxn[:, ds(32 * j + 16, 16)])
                n_t = work.tile([128, 16], f32, tag="n_t")
                nc.scalar.activation(n_t[:], u[:], AF.Tanh)

                # while ACT runs tanh: zc = 1-z, e0 = z*h
                zc = work.tile([128, 16], f32, tag="zc")
                nc.vector.tensor_scalar(zc[:], rz[:, 16:32], -1.0, 1.0,
                                        op0=OP.mult, op1=OP.add)
                e0 = work.tile([128, 2, B], f32, tag="e0")
                z_v = rz[:, 16:32].rearrange("p (c b) -> p c b", b=B)
                h_v = z8[:] if t == 0 else rhs[:, :, ds(roff, B)]
                nc.vector.tensor_mul(e0[:], z_v, h_v)

                m = work.tile([128, 16], f32, tag="m")
                nc.vector.tensor_mul(m[:], n_t[:], zc[:])
                # h' = n*(1-z) + z*h  (bf16, straight into history buffer)
                nc.vector.tensor_add(
                    hh[t // RPT][:, :, ds(j * B, B)],
                    m[:].rearrange("p (c b) -> p c b", b=B), e0[:]
                )

                # schedule next block's pre-fill + this block's projection
                if j == 7 and t // RPT + 1 < n_rt:
                    nrzp, ngxn, pf_q = emit_prefill(t // RPT + 1)
                    nxt = (nrzp, ngxn)
                for _ in range(3):
                    if pf_q:
                        pf_q.pop(0)()
                if j == RPT - 1:
                    emit_proj(t // RPT)
                    if nxt is not None:
                        rzp, gxn = nxt
                        nxt = None
                for _ in range(3):
                    if proj_q:
                        proj_q.pop(0)()

            if (tt_steps % RPT) != 0:
                emit_proj(n_rt - 1)
            while proj_q:
                proj_q.pop(0)()

    nc.compile()
    return nc


def _prep_inputs(seqs, emb, W_ih, W_hh, b_ih, b_hh, W_out, b_out, tt_steps=TT):
    """Host-side shard prep. Returns in_maps (one dict per core)."""
    seqs = np.asarray(seqs)
    emb = np.asarray(emb, dtype=np.float32)
    W_ih = np.asarray(W_ih, dtype=np.float32)
    W_hh = np.asarray(W_hh, dtype=np.float32)
    b_ih = np.asarray(b_ih, dtype=np.float32)
    b_hh = np.asarray(b_hh, dtype=np.float32)
    W_out = np.asarray(W_out, dtype=np.float32)
    b_out = np.asarray(b_out, dtype=np.float32)

    rows = tt_steps * B
    in_tokens = np.concatenate(
        [np.zeros((B, 1), dtype=seqs.dtype), seqs[:, : T - 2]], axis=1
    )[:, :tt_steps]                      # [B, tt]
    x = emb[in_tokens]                   # [B, tt, D]
    xT = np.ascontiguousarray(
        x.transpose(2, 1, 0).reshape(D, rows)
    )                                    # [D, t*8+b]
    xT_b = xT.reshape(2, 128, rows).astype(_bf16)

    wih_s = np.ascontiguousarray(W_ih.T).reshape(2, 128, 768).astype(_bf16)
    whh_s = np.ascontiguousarray(W_hh.T).reshape(2, 128, 768).astype(_bf16)

    brz = (b_ih[:512] + b_hh[:512]).reshape(1, 512).astype(_bf16)
    bhn = b_hh[512:].reshape(1, 256).astype(_bf16)
    bxn = b_ih[512:].reshape(1, 256).astype(_bf16)

    common = dict(xT=xT_b, wih=wih_s, whh=whh_s, brz=brz, bhn=bhn, bxn=bxn)
    in_maps = []
    for c in range(NCORES):
        wo = W_out[c * VS:(c + 1) * VS]                      # [VS, H]
        wo_t = np.ascontiguousarray(wo.T).reshape(2, 128, VS).astype(_bf16)
        bo = b_out[c * VS:(c + 1) * VS].reshape(1, VS).astype(_bf16)
        in_maps.append(dict(common, wout=wo_t, bout=bo))
    return in_maps


def run(inputs, tt_steps=TT, trace=False):
    """Run the kernel; returns (full_output, BassKernelResults)."""
    from concourse import bass_utils

    key = tt_steps
    if key not in _CACHE:
        _CACHE[key] = _build(tt_steps)
    nc = _CACHE[key]

    in_maps = _prep_inputs(
        inputs["seqs"], inputs["emb"], inputs["W_ih"], inputs["W_hh"],
        inputs["b_ih"], inputs["b_hh"], inputs["W_out"], inputs["b_out"],
        tt_steps=tt_steps,
    )
    res = bass_utils.run_bass_kernel_spmd(
        nc, in_maps, core_ids=list(range(NCORES)), trace=trace,
    )
    shards = [res.results[c]["out"] for c in range(NCORES)]   # [rows, VS] f32
    full = np.concatenate(shards, axis=1)                     # [rows, V]
    out = np.ascontiguousarray(
        full.reshape(tt_steps, B, V).transpose(1, 0, 2)
    ).astype(np.float32)
    return out, res


def kernel(labels, seqs, emb, W_ih, W_hh, b_ih, b_hh, W_out, b_out):
    out, _ = run(dict(seqs=seqs, emb=emb, W_ih=W_ih, W_hh=W_hh, b_ih=b_ih,
                      b_hh=b_hh, W_out=W_out, b_out=b_out))
    return out
